# revision 1
# baseline (speedup 1.0000x reference)
"""Multi-head attention kernel for Trainium2 (8 NeuronCores).

Problem: inputs query/key/value [2, 64, 64, 256] fp32, NHEAD=8, D=32.
reference: q,k,v -> [N=2, L=4096, H=8, D=32]; softmax(q.k^T/sqrt(D)) @ v.

Sharding: 16 (batch, head) pairs over 8 cores -> each core handles one
batch n = core//4 and two adjacent heads (2*hp, 2*hp+1), hp = core%4, so
its input slice is [4096, 64] contiguous channels.

Per-core algorithm (flash-style, S^T layout, no max subtraction --
logits are ~N(0,1) so exp() is well within fp32 range):
  Q^T, K^T [d=32, 4096] built via PE transposes of [128, 64] slabs.
  V' [s, 33] = [V | 1] (ones column -> softmax denominator for free).
  For each l-tile (512 cols) and each (head, s-tile) unit:
    MM1: S^T block [s=128, l=512]  = K^T_tile.T @ Q^T_tile      (PE)
    exp: ACT Exp(scale=1/sqrt(32)) PSUM -> SBUF, 3-bank groups  (ACT)
    MM2: O'^T [33, 512] += V'_tile.T @ expS^T_block             (PE, accum)
  Epilogue per l-tile: copy O'^T to SBUF, reciprocal of the denominator
  row, gpsimd partition-broadcast, multiply, DMA out.
"""

import numpy as np

L = 4096
D = 32
P = 128
NT = L // P            # 32 s-tiles per head
LT = 512               # l-tile width
N_LT = L // LT         # 8 l-tiles
GS = 2                 # (head, s-tile) units per ACT group (2 PSUM banks)
MM2_LAG = 9            # groups of AV matmuls held back (software pipeline depth)
TEMP = 1.0 / np.sqrt(np.float32(D))

_CACHE = {}


_MAXW = 1  # walrus codegen in this container allows 1 sem wait per instruction


def _split_waits_json(bir_json: bytes) -> bytes:
    """Rewrite BIR so no instruction carries more than _MAXW sem waits:
    excess waits move to EventSemaphore carrier instructions inserted
    immediately before, on the same engine (identical blocking semantics)."""
    import json

    m = json.loads(bir_json)
    ctr = 0
    for fn in m.get("functions", []):
        for blk in fn.get("blocks", []):
            out = []
            changed = False
            for ins in blk.get("instructions", []):
                si = ins.get("sync_info")
                waits = si.get("on_wait") if si else None
                if waits and len(waits) > _MAXW:
                    changed = True
                    excess = waits[: -_MAXW]
                    si["on_wait"] = waits[-_MAXW:]
                    for i in range(0, len(excess), _MAXW):
                        ctr += 1
                        out.append(
                            {
                                "debug": ins.get("debug", 0),
                                "engine": ins["engine"],
                                "ins": [],
                                "outs": [],
                                "name": f"EVW-{ctr}",
                                "opcode": "EventSemaphore",
                                "sync_info": {
                                    "on_wait": excess[i : i + _MAXW],
                                    "on_update": [],
                                },
                            }
                        )
                out.append(ins)
            if changed:
                blk["instructions"] = out
    return json.dumps(m).encode()


def _apply_drain_patch():
    """Hook compile_bir_kernel (both the native and the bass2jax/PJRT entry
    points) to run the wait-splitting BIR rewrite before walrus."""
    import concourse.bass_utils as bu

    if getattr(bu, "_ant_split_waits", False):
        return
    orig = bu.compile_bir_kernel

    def wrapped(bir_json, tmpdir, neff_name="file.neff"):
        return orig(_split_waits_json(bir_json), tmpdir, neff_name)

    bu.compile_bir_kernel = wrapped
    bu._ant_split_waits = True
    try:
        import concourse.bass2jax as b2j

        b2j.compile_bir_kernel = wrapped
    except ImportError:
        pass


def _build(mode="mixed"):
    """Build the per-core Bass program. mode: 'f32r' | 'bf16' | 'f32'."""
    import concourse.bass as bass
    import concourse.mybir as mybir
    import concourse.tile as tile
    from concourse.masks import make_identity

    _apply_drain_patch()

    f32 = mybir.dt.float32
    if mode == "bf16":
        sdt = mybir.dt.bfloat16       # storage dtype for QK^T matmul inputs
        tdt = mybir.dt.bfloat16       # dtype going through the PE transposes
        avdt = mybir.dt.bfloat16      # storage dtype for the AV side
    elif mode == "f32r":
        sdt = mybir.dt.float32r       # producers must emit rounded f32r
        tdt = f32
        avdt = mybir.dt.float32r
    elif mode == "mixed":
        # QK^T in f32r (logits are the accuracy-sensitive part); the AV
        # side in bf16 (errors in exp weights largely cancel between the
        # numerator and the shared denominator) -> 1 cycle/row matmuls.
        sdt = mybir.dt.float32r
        tdt = f32
        avdt = mybir.dt.bfloat16
    else:
        sdt = f32
        tdt = f32
        avdt = f32

    if mode == "bf16":
        gs, sp_bufs, lag = 2, 3, 9
    else:
        gs, sp_bufs, lag = 3, 2, 6

    nc = bass.Bass("TRN2", debug=False)
    q_d = nc.dram_tensor("q", [L, 64], f32, kind="ExternalInput")
    k_d = nc.dram_tensor("k", [L, 64], f32, kind="ExternalInput")
    v_d = nc.dram_tensor("v", [L, 64], f32, kind="ExternalInput")
    o_d = nc.dram_tensor("o", [L, 64], f32, kind="ExternalOutput")

    with tile.TileContext(nc) as tc:
        with (
            tc.tile_pool(name="const", bufs=1) as const_pool,
            tc.tile_pool(name="slab", bufs=1) as slab_pool,
            tc.tile_pool(name="persist", bufs=1) as persist_pool,
            tc.tile_pool(name="spsum", bufs=sp_bufs, space="PSUM") as spsum,
            tc.tile_pool(name="apsum", bufs=2, space="PSUM") as apsum,
            tc.tile_pool(name="exps", bufs=3 + lag) as exps_pool,
            tc.tile_pool(name="epil", bufs=3) as epil_pool,
        ):
            ident = const_pool.tile([P, P], tdt)
            make_identity(nc, ident)

            # ---- load q/k/v as [p, t, c] slabs -------------------------
            qs = slab_pool.tile([P, NT, 64], f32)
            ks = slab_pool.tile([P, NT, 64], f32)
            vs = slab_pool.tile([P, NT, 64], f32)
            # chunked loads: the first transposes/converts unblock after
            # ~1/4 of the transfer instead of the full 1MB per tensor
            for dst_t, src_t in ((qs, q_d), (ks, k_d), (vs, v_d)):
                src_ap = src_t.ap().rearrange("(t p) c -> p t c", p=P)
                for c4 in range(4):
                    ts_ = slice(c4 * 8, c4 * 8 + 8)
                    nc.sync.dma_start(out=dst_t[:, ts_, :], in_=src_ap[:, ts_, :])

            if mode == "bf16":
                qsb = slab_pool.tile([P, NT, 64], tdt)
                ksb = slab_pool.tile([P, NT, 64], tdt)
                for i in range(4):
                    s = slice(i * 8, i * 8 + 8)
                    nc.vector.tensor_copy(out=qsb[:, s, :], in_=qs[:, s, :])
                    nc.vector.tensor_copy(out=ksb[:, s, :], in_=ks[:, s, :])
            else:
                qsb, ksb = qs, ks

            # ---- V' = [v_h | 1] per head, per s-tile: [p, t, 66] -------
            # ones column LAST: O'^T row 32 of each head block = denominator
            # (numerator rows stay 32-partition aligned for DVE access)
            ones_f = const_pool.tile([P, 64], f32)
            nc.vector.memset(ones_f, 1.0)
            # V' = [v_h | 1] per head per s-tile: ones column LAST so the
            # denominator rides in the same M=33 matmul stream as the
            # numerator (no extra rhs streaming).
            vp = persist_pool.tile([P, NT, 66], avdt)
            nc.vector.tensor_copy(out=vp[:, :, 32:33], in_=ones_f[:, 0:NT])
            nc.vector.tensor_copy(out=vp[:, :, 65:66], in_=ones_f[:, 0:NT])
            for c4 in range(4):
                ts_ = slice(c4 * 8, c4 * 8 + 8)
                nc.vector.tensor_copy(
                    out=vp[:, ts_, 0:32], in_=vs[:, ts_, 0:32]
                )
                nc.vector.tensor_copy(
                    out=vp[:, ts_, 33:65], in_=vs[:, ts_, 32:64]
                )
            # row slices used by the K=1 reciprocal broadcast matmuls
            onesb = persist_pool.tile([P, 33], avdt)
            nc.vector.tensor_copy(out=onesb, in_=ones_f[:, 0:33])

            # ---- Q^T, K^T via PE transposes ----------------------------
            # [128, L]: strips 0/1 = head0/head1 d's, strips 2/3 replicate
            # them so four K=32 matmuls can run row-packed concurrently.
            qt = persist_pool.tile([P, L], sdt)
            kt = persist_pool.tile([P, L], sdt)
            for dst, src in ((qt, qsb), (kt, ksb)):
                for g in range(NT // 4):
                    # reuse the S-block PSUM slots for prologue transposes
                    tp = spsum.tile([64, 4 * P], tdt, tag="sp")
                    for j in range(4):
                        t = 4 * g + j
                        nc.tensor.transpose(
                            tp[:, j * P : (j + 1) * P], src[:, t, :], ident
                        )
                    nc.vector.tensor_copy(
                        out=dst[0:64, g * 512 : (g + 1) * 512], in_=tp
                    )
                    # replicate strips 0-1 into 2-3 incrementally so early
                    # s-tiles' strip-2/3 matmuls don't wait for the whole
                    # transpose prologue (SBUF->SBUF DMA crosses partitions)
                    nc.sync.dma_start(
                        out=dst[64:128, g * 512 : (g + 1) * 512],
                        in_=dst[0:64, g * 512 : (g + 1) * 512],
                    )

            # ---- main loop --------------------------------------------
            units = [(t, h) for t in range(NT) for h in (0, 1)]
            groups = [units[i : i + gs] for i in range(0, len(units), gs)]

            # Software-pipelined emission carried ACROSS l-tiles: MM2s for
            # s-tile t are emitted MM2_LAG groups after t's exp -- this
            # keeps MM1(g+1..g+L) ahead of MM2(g) in the in-order PE
            # stream, so ACT(g+1)'s wait on the PE tick after MM1(g+1)
            # does not transitively chain through MM2(g) -> ACT(g). Each
            # l-tile's epilogue is emitted as soon as its last MM2 flushes.
            accum_by_lt = {}
            ex_loc = {}
            pend = []

            def emit_epilogue(lt):
                lsl = slice(lt * LT, (lt + 1) * LT)
                accum = accum_by_lt.pop(lt)
                rec = epil_pool.tile([P, LT], avdt, tag="rec")
                with nc.allow_low_precision(
                    reason="softmax denominators are O(4096); rounding "
                    "the reciprocal to the matmul dtype is harmless"
                ):
                    nc.vector.reciprocal(
                        out=rec[32:33, :], in_=accum[32:33, :]
                    )
                    nc.vector.reciprocal(
                        out=rec[96:97, :], in_=accum[96:97, :]
                    )
                # bc rows 0-31 / 64-95 = broadcast reciprocal denominators
                # (K=1 matmuls from array rows 32 / 96); reuses an apsum
                # slot -- the accumulator is freed by the reciprocal reads
                bc = apsum.tile([P, LT], f32, tag="accum")
                nc.tensor.matmul(
                    bc[0:32, :], onesb[32:33, 0:32], rec[32:33, :],
                    start=True, stop=True, tile_position=(32, 0),
                )
                nc.tensor.matmul(
                    bc[64:96, :], onesb[96:97, 0:32], rec[96:97, :],
                    start=True, stop=True, tile_position=(96, 64),
                )
                oc = epil_pool.tile([P, LT], f32, tag="oc")
                nc.vector.tensor_copy(out=oc, in_=accum)
                o_n = epil_pool.tile([P, LT], f32, tag="o_n")
                nc.vector.tensor_mul(o_n[0:32, :], oc[0:32, :], bc[0:32, :])
                nc.vector.tensor_mul(
                    o_n[64:96, :], oc[64:96, :], bc[64:96, :]
                )
                # DVE 32x32 block transpose: o_t[:, 32b:32b+32] =
                # o_n[:, 32b:32b+32].T, i.e. partition p = l within
                # block, columns = d -> DRAM rows become 128B runs.
                o_t = epil_pool.tile([P, LT], f32, tag="o_t")
                nc.vector.transpose(out=o_t[0:32, :], in_=o_n[0:32, :])
                nc.vector.transpose(out=o_t[64:96, :], in_=o_n[64:96, :])
                for h in (0, 1):
                    nc.sync.dma_start(
                        out=o_d.ap()[lsl, 32 * h : 32 * h + 32].rearrange(
                            "(blk p) d -> p blk d", p=32
                        ),
                        in_=o_t[64 * h : 64 * h + 32, :].rearrange(
                            "p (blk d) -> p blk d", d=32
                        ),
                    )

            def flush_mm2(limit):
                while pend and len(pend) > limit:
                    lt, t, ex0, ex1 = pend.pop(0)
                    # one accumulator bank per l-tile: rows 0-31 h0
                    # numerator, 32-63 h1 numerator, 64 h0 denominator,
                    # 96 h1 denominator -- four col-packed matmuls per
                    # s-tile run concurrently.
                    if lt not in accum_by_lt:
                        accum = apsum.tile([P, LT], f32, tag="accum")
                        accum_by_lt[lt] = accum
                    accum = accum_by_lt[lt]
                    st_f = dict(start=(t == 0), stop=(t == NT - 1))
                    # rows 0-31 h0 numerator, 32 h0 denominator; rows
                    # 64-95 h1 numerator, 96 h1 denominator. Two M=33
                    # col-packed streams (strips {0,1} and {2,3}).
                    nc.tensor.matmul(
                        accum[0:33, :], vp[:, t, 0:33], ex0,
                        tile_position=(0, 0), **st_f,
                    )
                    nc.tensor.matmul(
                        accum[64:97, :], vp[:, t, 33:66], ex1,
                        tile_position=(0, 64), **st_f,
                    )
                    if t == NT - 1:
                        emit_epilogue(lt)

            def emit_group(lt, grp):
                lsl = slice(lt * LT, (lt + 1) * LT)
                w = len(grp) * LT
                sp = spsum.tile([P, gs * LT], f32, tag="sp")
                ex = exps_pool.tile([P, gs * LT], avdt, tag="ex")
                for j, (t, h) in enumerate(grp):
                    # row strip rotates over units so 4 consecutive K=32
                    # matmuls occupy disjoint 32-row groups of the array
                    # and execute concurrently
                    st = 32 * ((2 * t + h) % 4)
                    nc.tensor.matmul(
                        sp[:, j * LT : (j + 1) * LT],
                        kt[st : st + 32, t * P : (t + 1) * P],
                        qt[st : st + 32, lsl],
                        start=True,
                        stop=True,
                        tile_position=(st, 0),
                    )
                nc.scalar.activation(
                    ex[:, :w],
                    sp[:, :w],
                    mybir.ActivationFunctionType.Exp,
                    scale=float(TEMP),
                )
                for j, (t, h) in enumerate(grp):
                    ex_loc[(lt, t, h)] = ex[:, j * LT : (j + 1) * LT]
                while ex_loc:
                    klt, kt_ = min((a, b) for a, b, _ in ex_loc)
                    if (klt, kt_, 0) not in ex_loc or (klt, kt_, 1) not in ex_loc:
                        break
                    pend.append(
                        (
                            klt,
                            kt_,
                            ex_loc.pop((klt, kt_, 0)),
                            ex_loc.pop((klt, kt_, 1)),
                        )
                    )
                flush_mm2(lag)

            n_g = len(groups)
            for lt in range(N_LT):
                for gi, grp in enumerate(groups):
                    emit_group(lt, grp)
                    if lt == N_LT - 1 and gi > n_g - lag:
                        flush_mm2(max(0, n_g - 1 - gi))
            flush_mm2(0)
    return nc


def _get_nc(mode):
    if mode not in _CACHE:
        _CACHE[mode] = _build(mode)
    return _CACHE[mode]


def kernel(query, key, value, mode="mixed", trace=False):
    from concourse.bass_utils import run_bass_kernel_spmd

    q = np.ascontiguousarray(np.asarray(query, np.float32)).reshape(2, L, 256)
    k = np.ascontiguousarray(np.asarray(key, np.float32)).reshape(2, L, 256)
    v = np.ascontiguousarray(np.asarray(value, np.float32)).reshape(2, L, 256)

    nc = _get_nc(mode)
    in_maps = []
    for c in range(8):
        n, hp = divmod(c, 4)
        sl = slice(64 * hp, 64 * hp + 64)
        in_maps.append(
            {
                "q": np.ascontiguousarray(q[n, :, sl]),
                "k": np.ascontiguousarray(k[n, :, sl]),
                "v": np.ascontiguousarray(v[n, :, sl]),
            }
        )
    kwargs = {}
    if trace:
        kwargs = dict(trace=True)
    res = run_bass_kernel_spmd(nc, in_maps, core_ids=list(range(8)), **kwargs)
    out = np.zeros((2, L, 8, 32), np.float32)
    for c, r in enumerate(res.results):
        n, hp = divmod(c, 4)
        out[n, :, 2 * hp : 2 * hp + 2, :] = r["o"].reshape(L, 2, 32)
    if trace:
        return out, res
    return out



# revision 9
# speedup vs baseline: 1.2822x; 1.2822x over previous
"""Multi-head attention kernel for Trainium2 (8 NeuronCores).

Problem: inputs query/key/value [2, 64, 64, 256] fp32, NHEAD=8, D=32.
reference: q,k,v -> [N=2, L=4096, H=8, D=32]; softmax(q.k^T/sqrt(D)) @ v.

Sharding: 16 (batch, head) pairs over 8 cores -> each core handles one
batch n = core//4 and two adjacent heads (2*hp, 2*hp+1), hp = core%4, so
its input slice is [4096, 64] contiguous channels.

Per-core algorithm (flash-style, S^T layout, no max subtraction --
logits are ~N(0,1) so exp() is well within fp32 range):
  Q^T, K^T [d=32, 4096] f32r built via PE transposes of [128, 64] slabs.
  V' [s, 33] = [V | 1] bf16 (ones column -> softmax denominator free).
  Main loop: chunks of 2 units (s-tile t, heads 0+1), [128, 1024] PSUM:
    MM1: S^T = K^T.T @ Q^T (PE, K=32, 4-row-packed across chunks)
    exp: split between two engines by a fixed interleave pattern:
      - ACT: exact Exp (scale=1/sqrt(32)) PSUM -> SBUF bf16
      - DVE: 1-op Schraudolph: y = x*c1 + (2^23*1.5 + bf16_bias); the
        f32 RNE add leaves round(x*c1)+bias in the mantissa, so the LOW
        16 bits of each f32 ARE the bf16 approx of exp(x*temp). MM2
        reads them via bitcast + stride-2 AP. (~2% per-element noise,
        averages out over 4096-term softmax rows; measured end-to-end
        rel err ~9e-3 even at 100% DVE.)
    MM2: O'^T [33, 512] += V'.T @ expS^T (PE, accum, 2 col-packed M=33)
  Epilogue per l-tile: denominator rows DMA-packed [1,512]->[128,4],
  one small DVE reciprocal, DMA-unpacked, K=1 matmul broadcast, one
  DVE multiply + 32x32 block transpose, DMA out.
"""

import numpy as np

L = 4096
D = 32
P = 128
NT = L // P            # 32 s-tiles per head
LT = 512               # l-tile width
N_LT = L // LT         # 8 l-tiles
TEMP = 1.0 / np.sqrt(np.float32(D))

# Schraudolph-in-bf16 constants for the DVE exp path (see module docstring)
C1 = float(128.0 * np.log2(np.e) * TEMP)
SHIFT = 7.0                       # mean-centering of the (1+f) vs 2^f error
C2 = float(12582912.0 + 127.0 * 128.0 - SHIFT)
# the residual mean log-error of the DVE path after SHIFT centering;
# applied as a free bias on the ACT path so both engines' weights match
ACT_BIAS = float(0.039721 - SHIFT * np.log(2.0) / 128.0)

_CACHE = {}


_MAXW = 1  # walrus codegen in this container allows 1 sem wait per instruction


def _split_waits_json(bir_json: bytes) -> bytes:
    """Rewrite BIR so no instruction carries more than _MAXW sem waits:
    excess waits move to EventSemaphore carrier instructions inserted
    immediately before, on the same engine (identical blocking semantics)."""
    import json

    m = json.loads(bir_json)
    ctr = 0
    for fn in m.get("functions", []):
        for blk in fn.get("blocks", []):
            out = []
            changed = False
            for ins in blk.get("instructions", []):
                si = ins.get("sync_info")
                waits = si.get("on_wait") if si else None
                if waits and len(waits) > _MAXW:
                    changed = True
                    excess = waits[: -_MAXW]
                    si["on_wait"] = waits[-_MAXW:]
                    for i in range(0, len(excess), _MAXW):
                        ctr += 1
                        out.append(
                            {
                                "debug": ins.get("debug", 0),
                                "engine": ins["engine"],
                                "ins": [],
                                "outs": [],
                                "name": f"EVW-{ctr}",
                                "opcode": "EventSemaphore",
                                "sync_info": {
                                    "on_wait": excess[i : i + _MAXW],
                                    "on_update": [],
                                },
                            }
                        )
                out.append(ins)
            if changed:
                blk["instructions"] = out
    return json.dumps(m).encode()


def _apply_drain_patch():
    """Hook compile_bir_kernel (both the native and the bass2jax/PJRT entry
    points) to run the wait-splitting BIR rewrite before walrus."""
    import concourse.bass_utils as bu

    if getattr(bu, "_ant_split_waits", False):
        return
    orig = bu.compile_bir_kernel

    def wrapped(bir_json, tmpdir, neff_name="file.neff"):
        return orig(_split_waits_json(bir_json), tmpdir, neff_name)

    bu.compile_bir_kernel = wrapped
    bu._ant_split_waits = True
    try:
        import concourse.bass2jax as b2j

        b2j.compile_bir_kernel = wrapped
    except ImportError:
        pass


def _build(dve_num=9, dve_den=20, lag=10, head_act=6):
    """Build the per-core Bass program.

    dve_num/dve_den: fraction of exp chunks handled by the DVE
    Schraudolph path (rest go to ACT exact exp). lag: number of chunks
    MM2 emission trails exp emission (software pipeline depth).
    head_act: first chunks forced to ACT while DVE does V' setup.
    """
    import concourse.bass as bass
    import concourse.mybir as mybir
    import concourse.tile as tile
    from concourse.masks import make_identity

    _apply_drain_patch()

    f32 = mybir.dt.float32
    f32r = mybir.dt.float32r
    bf16 = mybir.dt.bfloat16
    AT = mybir.ActivationFunctionType
    ALU = mybir.AluOpType

    nc = bass.Bass("TRN2", debug=False)
    q_d = nc.dram_tensor("q", [L, 64], f32, kind="ExternalInput")
    k_d = nc.dram_tensor("k", [L, 64], f32, kind="ExternalInput")
    v_d = nc.dram_tensor("v", [L, 64], f32, kind="ExternalInput")
    o_d = nc.dram_tensor("o", [L, 64], f32, kind="ExternalOutput")

    def is_dve_chunk(g):
        if g < head_act:
            return False
        return ((g - head_act) * dve_num) % dve_den < dve_num

    with tile.TileContext(nc) as tc:
        with (
            tc.tile_pool(name="const", bufs=1) as const_pool,
            tc.tile_pool(name="slab", bufs=1) as slab_pool,
            tc.tile_pool(name="persist", bufs=1) as persist_pool,
            tc.tile_pool(name="spsum", bufs=3, space="PSUM") as spsum,
            tc.tile_pool(name="apsum", bufs=2, space="PSUM") as apsum,
            tc.tile_pool(name="exa", bufs=4 + lag) as exa_pool,
            tc.tile_pool(name="exd", bufs=4 + lag) as exd_pool,
            tc.tile_pool(name="epil", bufs=3) as epil_pool,
        ):
            ident = const_pool.tile([P, P], f32)
            make_identity(nc, ident)
            bias_t = const_pool.tile([P, 1], f32)
            nc.vector.memset(bias_t, float(ACT_BIAS))

            # defined values in the apsum slots so whole-tile epilogue
            # reads (rows the MM2s never write) are well-defined
            pz0 = apsum.tile([P, LT], f32, tag="accum")
            pz1 = apsum.tile([P, LT], f32, tag="accum")
            nc.vector.memset(pz0, 0.0)
            nc.vector.memset(pz1, 0.0)

            # ---- load q/k as [p, t, c] slabs, k/q interleaved ----------
            qs = slab_pool.tile([P, NT, 64], f32)
            ks = slab_pool.tile([P, NT, 64], f32)
            vs = slab_pool.tile([P, NT, 64], f32)
            q_ap = q_d.ap().rearrange("(t p) c -> p t c", p=P)
            k_ap = k_d.ap().rearrange("(t p) c -> p t c", p=P)
            v_ap = v_d.ap().rearrange("(t p) c -> p t c", p=P)
            for c8 in range(4):
                ts_ = slice(c8 * 8, c8 * 8 + 8)
                nc.sync.dma_start(out=ks[:, ts_, :], in_=k_ap[:, ts_, :])
                nc.sync.dma_start(out=qs[:, ts_, :], in_=q_ap[:, ts_, :])
            for c16 in range(2):
                ts_ = slice(c16 * 16, c16 * 16 + 16)
                nc.sync.dma_start(out=vs[:, ts_, :], in_=v_ap[:, ts_, :])

            # ---- Q^T, K^T via PE transposes, k/q interleaved -----------
            # [128, L]: strips 0/1 = head0/head1 d's, strips 2/3 replicate
            # them so four K=32 matmuls can run row-packed concurrently.
            # PSUM->SBUF copies alternate ACT/DVE to halve the prologue.
            qt = persist_pool.tile([P, L], f32r)
            kt = persist_pool.tile([P, L], f32r)
            for g in range(NT // 4):
                for dst, src in ((kt, ks), (qt, qs)):
                    tp = spsum.tile([64, 4 * P], f32, tag="sp")
                    for j in range(4):
                        t = 4 * g + j
                        nc.tensor.transpose(
                            tp[:, j * P : (j + 1) * P], src[:, t, :], ident
                        )
                    dsl = dst[0:64, g * 512 : (g + 1) * 512]
                    if dst is kt:
                        nc.scalar.copy(dsl, tp)
                    else:
                        nc.vector.tensor_copy(out=dsl, in_=tp)
                    nc.sync.dma_start(
                        out=dst[64:128, g * 512 : (g + 1) * 512],
                        in_=dst[0:64, g * 512 : (g + 1) * 512],
                    )

            # ---- V' = [v_h | 1] per head, per s-tile: [p, t, 66] -------
            # ones column LAST in each head's 33-wide block: O'^T row
            # 32/96 = softmax denominator for free.
            vp = persist_pool.tile([P, NT, 66], bf16)
            ones_f = const_pool.tile([P, 64], f32)
            nc.vector.memset(ones_f, 1.0)
            nc.vector.memset(vp[:, :, 32:33], 1.0)
            nc.vector.memset(vp[:, :, 65:66], 1.0)
            for c8 in range(4):
                ts_ = slice(c8 * 8, c8 * 8 + 8)
                nc.vector.tensor_copy(out=vp[:, ts_, 0:32], in_=vs[:, ts_, 0:32])
                nc.vector.tensor_copy(out=vp[:, ts_, 33:65], in_=vs[:, ts_, 32:64])
            # K=1 lhsT for the reciprocal-broadcast matmuls (bf16: fp32-family
            # matmuls fail the ISA dst-partition check at col position 64)
            onesb = persist_pool.tile([P, 32], bf16)
            nc.vector.tensor_copy(out=onesb, in_=ones_f[:, 0:32])

            # ---- main loop --------------------------------------------
            accum_by_lt = {}
            pend = []

            def emit_epilogue(lt):
                lsl = slice(lt * LT, (lt + 1) * LT)
                accum = accum_by_lt.pop(lt)
                # pack the two denominator rows [1,512] -> [128,4] each so
                # the (8-cycle-per-element) reciprocal runs on FD=8
                oc = epil_pool.tile([P, LT], f32, tag="oc")
                nc.scalar.copy(oc, accum)
                dp = epil_pool.tile([P, 8], f32, tag="dp")
                rp = epil_pool.tile([P, 8], f32, tag="rp")
                for h in (0, 1):
                    nc.sync.dma_start(
                        out=dp[:, 4 * h : 4 * h + 4],
                        in_=oc[32 + 64 * h : 33 + 64 * h, :],
                    )
                nc.vector.reciprocal(out=rp, in_=dp)
                rpb = epil_pool.tile([P, 8], bf16, tag="rpb")
                with nc.allow_low_precision(
                    reason="softmax denominators are O(4096); bf16 "
                    "reciprocals cost ~0.2% common-mode on the output"
                ):
                    nc.vector.tensor_copy(out=rpb, in_=rp)
                # rec rows live at partitions 32 / 96 to feed the K=1
                # broadcast matmuls
                rec = epil_pool.tile([P, LT], bf16, tag="rec")
                for h in (0, 1):
                    nc.sync.dma_start(
                        out=rec[32 + 64 * h : 33 + 64 * h, :],
                        in_=rpb[:, 4 * h : 4 * h + 4],
                    )
                # bc rows 0-31 / 64-95 = broadcast reciprocal denominators
                bc = apsum.tile([P, LT], f32, tag="accum")
                nc.tensor.matmul(
                    bc[0:32, :], onesb[32:33, :], rec[32:33, :],
                    start=True, stop=True, tile_position=(32, 0),
                )
                nc.tensor.matmul(
                    bc[64:96, :], onesb[96:97, :], rec[96:97, :],
                    start=True, stop=True, tile_position=(96, 64),
                )
                o_n = epil_pool.tile([P, LT], f32, tag="o_n")
                nc.vector.tensor_mul(o_n, oc, bc)
                # DVE 32x32 block transpose -> DRAM rows become 128B runs
                o_t = epil_pool.tile([P, LT], f32, tag="o_t")
                nc.vector.transpose(out=o_t, in_=o_n)
                for h in (0, 1):
                    nc.sync.dma_start(
                        out=o_d.ap()[lsl, 32 * h : 32 * h + 32].rearrange(
                            "(blk p) d -> p blk d", p=32
                        ),
                        in_=o_t[64 * h : 64 * h + 32, :].rearrange(
                            "p (blk d) -> p blk d", d=32
                        ),
                    )

            def flush_mm2(limit):
                while pend and len(pend) > limit:
                    lt, t, ex0, ex1 = pend.pop(0)
                    if lt not in accum_by_lt:
                        accum_by_lt[lt] = apsum.tile(
                            [P, LT], f32, tag="accum", name="accum"
                        )
                    accum = accum_by_lt[lt]
                    st_f = dict(start=(t == 0), stop=(t == NT - 1))
                    # rows 0-31 h0 numerator, 32 h0 denominator; rows
                    # 64-95 h1 numerator, 96 h1 denominator.
                    nc.tensor.matmul(
                        accum[0:33, :], vp[:, t, 0:33], ex0,
                        tile_position=(0, 0), **st_f,
                    )
                    nc.tensor.matmul(
                        accum[64:97, :], vp[:, t, 33:66], ex1,
                        tile_position=(0, 64), **st_f,
                    )
                    if t == NT - 1:
                        emit_epilogue(lt)

            def emit_chunk(lt, t, g):
                lsl = slice(lt * LT, (lt + 1) * LT)
                sp = spsum.tile([P, 2 * LT], f32, tag="sp")
                for h in (0, 1):
                    # row strip rotates over units so 4 consecutive K=32
                    # matmuls occupy disjoint 32-row groups of the array
                    st = 32 * ((2 * t + h) % 4)
                    nc.tensor.matmul(
                        sp[:, h * LT : (h + 1) * LT],
                        kt[st : st + 32, t * P : (t + 1) * P],
                        qt[st : st + 32, lsl],
                        start=True,
                        stop=True,
                        tile_position=(st, 0),
                    )
                if is_dve_chunk(g):
                    exd = exd_pool.tile([P, 2 * LT], f32, tag="exd")
                    nc.vector.tensor_scalar(
                        exd, sp, C1, C2, ALU.mult, ALU.add
                    )
                    exb = exd.bitcast(bf16)
                    ex0 = exb[:, 0 * LT : 2 * LT : 2]
                    ex1 = exb[:, 2 * LT : 4 * LT : 2]
                else:
                    exa = exa_pool.tile([P, 2 * LT], bf16, tag="exa")
                    nc.scalar.activation(
                        exa, sp, AT.Exp, scale=float(TEMP), bias=bias_t[:, 0:1]
                    )
                    ex0 = exa[:, 0:LT]
                    ex1 = exa[:, LT : 2 * LT]
                pend.append((lt, t, ex0, ex1))
                flush_mm2(lag)

            n_g = NT  # chunks per l-tile
            for lt in range(N_LT):
                for t in range(NT):
                    g = lt * NT + t
                    emit_chunk(lt, t, g)
                    if lt == N_LT - 1 and t > n_g - lag:
                        flush_mm2(max(0, n_g - 1 - t))
            flush_mm2(0)
    return nc


def _get_nc(params):
    if params not in _CACHE:
        _CACHE[params] = _build(*params)
    return _CACHE[params]


def kernel(query, key, value, dve_num=9, dve_den=20, lag=10, head_act=6,
           trace=False):
    from concourse.bass_utils import run_bass_kernel_spmd

    q = np.ascontiguousarray(np.asarray(query, np.float32)).reshape(2, L, 256)
    k = np.ascontiguousarray(np.asarray(key, np.float32)).reshape(2, L, 256)
    v = np.ascontiguousarray(np.asarray(value, np.float32)).reshape(2, L, 256)

    nc = _get_nc((dve_num, dve_den, lag, head_act))
    in_maps = []
    for c in range(8):
        n, hp = divmod(c, 4)
        sl = slice(64 * hp, 64 * hp + 64)
        in_maps.append(
            {
                "q": np.ascontiguousarray(q[n, :, sl]),
                "k": np.ascontiguousarray(k[n, :, sl]),
                "v": np.ascontiguousarray(v[n, :, sl]),
            }
        )
    kwargs = {}
    if trace:
        kwargs = dict(trace=True)
    res = run_bass_kernel_spmd(nc, in_maps, core_ids=list(range(8)), **kwargs)
    out = np.zeros((2, L, 8, 32), np.float32)
    for c, r in enumerate(res.results):
        n, hp = divmod(c, 4)
        out[n, :, 2 * hp : 2 * hp + 2, :] = np.asarray(
            r["o"], np.float32
        ).reshape(L, 2, 32)
    if trace:
        return out, res
    return out
